# revision 26
# baseline (speedup 1.0000x reference)
"""Trainium2 Bass kernel for BasicQuantumAttention.

Contract: kernel(**inputs) takes the FULL (unsharded) numpy inputs of the
reference problem (B=4, L=2048, D=512) and returns the full output
(out_real, out_imag), each [B, L, D] float32.

Sharding: 8 NeuronCores; core c handles batch b=c//2, query half h=c%2
(1024 queries). Each core computes the fused QKV projection only for its
own 1024 rows; the key/value projections are then exchanged within the
core pair via an in-kernel pairwise AllGather, so no projection work is
duplicated. Key order is global (rows 0:2047 of the batch) on both cores
of a pair, which keeps the SPMD program identical on every core.

Layouts (all matmuls bf16, f32 PSUM accumulation):
  - x is passed transposed per core: xT [6D=3072, own 1024 rows].
  - q,k projections are computed weight-stationary into ^T layout
    [feat, row]; v is computed into row-major [row, feat] layout.
  - Only the qkv output blocks the reference actually uses are computed
    (q_real, k_real, v_real, v_imag) -- 2/3 of the fused projection.
  - scores^T [key, query] = (k^T tile).T @ q^T; the per-key padding mask
    and the 1/sqrt(D) scale fold into the ACT Exp (bias/scale).
  - attn^T tiles directly feed Z^T accumulation; an M=1 ones-matmul
    produces the softmax row sums; normalization is deferred to after
    the output projection (diag scaling commutes with the right-matmul),
    and b_out is added in the same fused DVE op.
"""

import numpy as np
import ml_dtypes

B, L, D = 4, 2048, 512
P = 128
IN_F = 6 * D          # 3072 input features of the fused projection
QK_F = 2 * D          # selected output features: q_real block + k_real block
V_F = 2 * D           # selected output features: v_real block + v_imag block
KT = IN_F // P        # 24 contraction tiles
NCORES = 8
HALF = L // 2         # 1024 rows owned per core
SCALE = float(D) ** -0.5
NEG = -30000.0        # additive key mask (exp underflows to exactly 0)
GROUPS = [[0, 1], [2, 3], [4, 5], [6, 7]]
NSEND = 12            # blocks of [128, HALF] sent to the pair: 4 k^T + 8 v

_NC_CACHE = {}


def _build_program(reps=1):
    import os
    import concourse.bass as bass
    import concourse.bacc as bacc
    import concourse.mybir as mybir
    import concourse.tile as tile
    from contextlib import ExitStack

    # Timing-ablation switch: skip the pair exchanges and read back own data
    # (incorrect results, identical instruction shape otherwise).
    NO_COLL = bool(os.environ.get("KERNEL_NO_COLL"))

    f32 = mybir.dt.float32
    bf16 = mybir.dt.bfloat16
    f8 = mybir.dt.float8e4
    DR = mybir.MatmulPerfMode.DoubleRow
    AF = mybir.ActivationFunctionType
    ALU = mybir.AluOpType
    PSUM = bass.MemorySpace.PSUM

    nc = bacc.Bacc(
        "TRN2",
        debug=False,
        enable_asserts=False,
        target_bir_lowering=False,
        num_devices=NCORES,
    )

    xT_d = nc.dram_tensor("xT", [IN_F, HALF], bf16, kind="ExternalInput").ap()
    wqk_d = nc.dram_tensor("wqkT", [IN_F, QK_F], bf16, kind="ExternalInput").ap()
    wv_d = nc.dram_tensor("wvT", [IN_F, V_F], bf16, kind="ExternalInput").ap()
    wo_d = nc.dram_tensor("woT", [V_F, V_F], bf16, kind="ExternalInput").ap()
    mb_d = nc.dram_tensor("maskb", [P, L // P], f32, kind="ExternalInput").ap()
    bqk_d = nc.dram_tensor("bqk", [P, QK_F // P], f32, kind="ExternalInput").ap()
    bvb_d = nc.dram_tensor("bvb", [P, V_F], f32, kind="ExternalInput").ap()
    bob_d = nc.dram_tensor("bob", [P, V_F], f32, kind="ExternalInput").ap()
    y_d = nc.dram_tensor("y", [HALF, V_F], f32, kind="ExternalOutput").ap()

    k_send = nc.dram_tensor("k_send", [4 * P, HALF], bf16).ap()
    k_recv = nc.dram_tensor("k_recv", [8 * P, HALF], bf16).ap()
    v_send0 = nc.dram_tensor("v_send0", [8 * P, 512], bf16).ap()
    v_recv0 = nc.dram_tensor("v_recv0", [16 * P, 512], bf16).ap()
    v_send1 = nc.dram_tensor("v_send1", [8 * P, 512], bf16).ap()
    v_recv1 = nc.dram_tensor("v_recv1", [16 * P, 512], bf16).ap()

    xT_r = xT_d.rearrange("(t p) n -> t p n", p=P)
    wqk_r = wqk_d.rearrange("(t p) n -> t p n", p=P)
    wv_r = wv_d.rearrange("(t p) n -> t p n", p=P)
    wo_r = wo_d.rearrange("(t p) n -> t p n", p=P)
    ksend_r = k_send.rearrange("(i p) n -> i p n", p=P)
    krecv_r = k_recv.rearrange("(i p) n -> i p n", p=P)
    vsend_r = [v.rearrange("(i p) n -> i p n", p=P) for v in (v_send0, v_send1)]
    vrecv_r = [v.rearrange("(i p) n -> i p n", p=P) for v in (v_recv0, v_recv1)]

    def _emit_body(tc, ctx):
        const = ctx.enter_context(tc.tile_pool(name="const", bufs=1))
        persist = ctx.enter_context(tc.tile_pool(name="persist", bufs=1))

        mb = const.tile([P, L // P], f32, tag="mb")
        nc.sync.dma_start(mb, mb_d)
        bqk = const.tile([P, QK_F // P], f32, tag="bqk")
        nc.sync.dma_start(bqk, bqk_d)
        ones_c = const.tile([P, 1], bf16, tag="ones_c")
        nc.vector.memset(ones_c, 1.0)
        ident1 = const.tile([1, 1], f32, tag="ident1")
        nc.vector.memset(ident1, 1.0)

        # Free-dim biases come pre-broadcast from the host (loaded after the
        # critical projection streams are queued).
        bvb = persist.tile([P, V_F], f32, tag="bvb")
        bob = persist.tile([P, V_F], f32, tag="bob")

        # Persistent attention operands + output-projection weights.
        q_sb = [persist.tile([P, HALF], bf16, tag=f"q{m}", name=f"q{m}") for m in range(4)]
        k_sb = [persist.tile([P, L], bf16, tag=f"k{m}", name=f"k{m}") for m in range(4)]
        v_sb = [persist.tile([P, V_F], bf16, tag=f"v{rb}", name=f"v{rb}") for rb in range(L // P)]
        wo_sb = [persist.tile([P, V_F], bf16, tag=f"wo{t}", name=f"wo{t}") for t in range(V_F // P)]

        # Staging tiles for the pair exchange (own k^T blocks + own v rows).
        kst = [persist.tile([P, HALF], bf16, tag=f"kst{m}", name=f"kst{m}") for m in range(4)]

        # Phase order: k-proj -> AG(k) -> v-proj (c0 -> AG(v0), c1 -> AG(v1))
        # -> q-proj -> attention.  Each exchange gets 40-120us of projection
        # matmuls as cover; readback DMAs are issued late so a pending
        # collective can never head-of-line-block a weight stream.
        with (
            tc.tile_pool(name="xp", bufs=1) as xp,
            tc.tile_pool(name="ws", bufs=9) as ws,
            tc.tile_pool(name="vstp", bufs=2) as vstp,
        ):
            x = []
            wk = []
            for k in range(KT):
                xt = xp.tile([P, HALF], bf16, tag=f"x{k}")
                nc.sync.dma_start(xt, xT_r[k])
                x.append(xt)
                wkt = ws.tile([P, 512], bf16, tag="wst", name=f"wk{k}")
                nc.sync.dma_start(wkt, wqk_r[k, :, 512:])
                wk.append(wkt)
            nc.sync.dma_start(bvb, bvb_d)

            # ---- k_real projection (^T layout), single pass, 8 PSUM banks.
            # PSUM accumulation is order-independent, so the last TAIL
            # k-steps are emitted chain-major: chain m finishes TAIL*(3-m)
            # k-steps early and its ACT drain + send DMA overlap the
            # remaining matmuls instead of serializing at the phase edge.
            TAIL = 3
            with tc.tile_pool(name="psk", bufs=1, space=PSUM) as psk:
                pks = [psk.tile([P, HALF], f32, tag=f"pk{m}", name=f"pk{m}")
                       for m in range(4)]
                for k in range(KT - TAIL):
                    for m in range(4):
                        for c in range(2):
                            nc.tensor.matmul(
                                pks[m][:, c * 512:(c + 1) * 512],
                                wk[k][:, m * P:(m + 1) * P],
                                x[k][:, c * 512:(c + 1) * 512],
                                start=(k == 0), stop=False,
                            )
                for m in range(4):
                    for k in range(KT - TAIL, KT):
                        for c in range(2):
                            nc.tensor.matmul(
                                pks[m][:, c * 512:(c + 1) * 512],
                                wk[k][:, m * P:(m + 1) * P],
                                x[k][:, c * 512:(c + 1) * 512],
                                start=False, stop=(k == KT - 1),
                            )
                    nc.scalar.activation(kst[m], pks[m], AF.Identity, bias=bqk[:, 4 + m:5 + m])
                    nc.sync.dma_start(ksend_r[m], kst[m])

            # Pairwise exchange of the k^T blocks (ranks are [even, odd] =
            # [global first half, global second half] for both pair members);
            # hidden behind the v and q projections that follow.
            if not NO_COLL:
                nc.gpsimd.collective_compute(
                    "AllGather", mybir.AluOpType.bypass,
                    replica_groups=GROUPS,
                    ins=[k_send.opt()], outs=[k_recv.opt()],
                )

            # ---- v_real|v_imag projection, row-major, c-outer (wv loaded
            # once); each 512-feature half is exchanged as soon as it's done.
            for c in range(2):
                with tc.tile_pool(name="psv", bufs=1, space=PSUM) as psv:
                    pv = [psv.tile([P, 512], f32, tag=f"pv{j}", name=f"pv{j}")
                          for j in range(8)]
                    wv_sl = []
                    for k in range(KT):
                        wsl = ws.tile([P, 512], bf16, tag="wst", name=f"wv{c}_{k}")
                        nc.sync.dma_start(wsl, wv_r[k, :, c * 512:(c + 1) * 512])
                        wv_sl.append(wsl)
                        if k >= KT - TAIL:
                            continue
                        for j in range(8):
                            nc.tensor.matmul(
                                pv[j], x[k][:, j * P:(j + 1) * P], wsl,
                                start=(k == 0), stop=False,
                            )
                    for j in range(8):
                        for k in range(KT - TAIL, KT):
                            nc.tensor.matmul(
                                pv[j], x[k][:, j * P:(j + 1) * P], wv_sl[k],
                                start=False, stop=(k == KT - 1),
                            )
                        vs = vstp.tile([P, 512], bf16, tag=f"vst{j}", name=f"vst{c}_{j}")
                        nc.vector.tensor_tensor(
                            vs, pv[j], bvb[:, c * 512:(c + 1) * 512], op=ALU.add,
                        )
                        nc.sync.dma_start(vsend_r[c][j], vs)
                if not NO_COLL:
                    nc.gpsimd.collective_compute(
                        "AllGather", mybir.AluOpType.bypass,
                        replica_groups=GROUPS,
                        ins=[(v_send0 if c == 0 else v_send1).opt()],
                        outs=[(v_recv0 if c == 0 else v_recv1).opt()],
                    )
                if c == 1:
                    # v1 readback on the gpsimd issue queue: it is busy until
                    # AG(v1) completes there anyway, so the readback starts
                    # the moment its data lands, blocking nothing.
                    for t in range(L // P):
                        nc.gpsimd.dma_start(
                            v_sb[t][:, 512:1024],
                            vsend_r[1][t % 8] if NO_COLL else vrecv_r[1][t],
                        )
                if c == 0:
                    # k readback on SP here: AG(k) has had the whole v c0
                    # pass as cover, so this does not block the SP queue; the
                    # inter-collective Pool queue stays free so AG(v0)/AG(v1)
                    # issue the moment their sends complete.
                    for hh in range(2):
                        for m in range(4):
                            nc.sync.dma_start(
                                k_sb[m][:, hh * HALF:(hh + 1) * HALF],
                                ksend_r[m] if NO_COLL else krecv_r[hh * 4 + m],
                            )
                    nc.sync.dma_start(bob, bob_d)
                    for t in range(V_F // P):
                        nc.sync.dma_start(wo_sb[t], wo_r[t])

            # ---- q_real projection (^T layout), single pass, 8 PSUM banks;
            # overlaps both exchanges.
            with tc.tile_pool(name="psq", bufs=1, space=PSUM) as psq:
                pq = [psq.tile([P, HALF], f32, tag=f"pq{m}", name=f"pq{m}")
                      for m in range(4)]
                wq_sl = []
                for k in range(KT):
                    wsl = ws.tile([P, 512], bf16, tag="wst", name=f"wq{k}")
                    nc.sync.dma_start(wsl, wqk_r[k, :, 0:512])
                    wq_sl.append(wsl)
                    if k >= KT - TAIL:
                        continue
                    for m in range(4):
                        for c in range(2):
                            nc.tensor.matmul(
                                pq[m][:, c * 512:(c + 1) * 512],
                                wsl[:, m * P:(m + 1) * P],
                                x[k][:, c * 512:(c + 1) * 512],
                                start=(k == 0), stop=False,
                            )
                # Staggered tails; the ch0 half of each bias ACT is emitted
                # first so the first scores chain only waits ~0.4us.
                for m in range(4):
                    for k in range(KT - TAIL, KT):
                        for c in range(2):
                            nc.tensor.matmul(
                                pq[m][:, c * 512:(c + 1) * 512],
                                wq_sl[k][:, m * P:(m + 1) * P],
                                x[k][:, c * 512:(c + 1) * 512],
                                start=False, stop=(k == KT - 1),
                            )
                    nc.scalar.activation(q_sb[m][:, 0:512], pq[m][:, 0:512],
                                         AF.Identity, bias=bqk[:, m:m + 1])
                for m in range(4):
                    nc.scalar.activation(q_sb[m][:, 512:HALF], pq[m][:, 512:HALF],
                                         AF.Identity, bias=bqk[:, m:m + 1])

            # v0 readback on SP after every weight stream is queued: AG(v0)
            # completed during the v c1 pass, so this drains immediately and
            # only the late y-output DMAs share its completion lanes.
            for t in range(L // P):
                nc.sync.dma_start(
                    v_sb[t][:, 0:512],
                    vsend_r[0][t % 8] if NO_COLL else vrecv_r[0][t],
                )

        # --------------------------- attention ---------------------------
        with (
            tc.tile_pool(name="at", bufs=1) as atp,
            tc.tile_pool(name="zs", bufs=2) as zsp,
            tc.tile_pool(name="ys", bufs=2) as ysp,
            tc.tile_pool(name="sm", bufs=2) as smp,
            tc.tile_pool(name="pssc", bufs=2, space=PSUM) as pssc,
            tc.tile_pool(name="psz", bufs=2, space=PSUM) as psz,
            tc.tile_pool(name="pssum", bufs=1, space=PSUM) as pssum,
            tc.tile_pool(name="pstp", bufs=1, space=PSUM) as pstp,
            tc.tile_pool(name="psy", bufs=1, space=PSUM) as psy,
        ):
            for ch in range(2):
                qs = slice(ch * 512, (ch + 1) * 512)

                # scores^T + masked exp, per key tile
                at = []
                for t in range(L // P):
                    ps = pssc.tile([P, 512], f32, tag="sc")
                    for d in range(4):
                        nc.tensor.matmul(
                            ps, k_sb[d][:, t * P:(t + 1) * P], q_sb[d][:, qs],
                            start=(d == 0), stop=(d == 3),
                        )
                    a = atp.tile([P, 512], bf16, tag=f"at{t}")
                    nc.scalar.activation(
                        a, ps, AF.Exp, bias=mb[:, t:t + 1], scale=SCALE
                    )
                    at.append(a)

                # softmax row-sums: ones^T @ attn^T, accumulated over key tiles
                sp = pssum.tile([1, 512], f32, tag="sum")
                for t in range(L // P):
                    nc.tensor.matmul(
                        sp, ones_c, at[t], start=(t == 0), stop=(t == L // P - 1)
                    )
                sums = smp.tile([1, 512], f32, tag="sums")
                nc.vector.tensor_copy(sums, sp)
                rc = []
                for s in range(4):
                    tp = pstp.tile([P, 1], f32, tag="tp")
                    nc.tensor.transpose(tp, sums[0:1, s * P:(s + 1) * P], ident1)
                    r = smp.tile([P, 1], f32, tag=f"rc{s}", name=f"rc{s}")
                    nc.vector.reciprocal(r, tp)
                    rc.append(r)

                # Z^T = sum_key v[key, dblk] * attn^T[key, q]
                z_sb = []
                for dblk in range(V_F // P):
                    pz = psz.tile([P, 512], f32, tag="z")
                    for t in range(L // P):
                        nc.tensor.matmul(
                            pz, v_sb[t][:, dblk * P:(dblk + 1) * P], at[t],
                            start=(t == 0), stop=(t == L // P - 1),
                        )
                    z = zsp.tile([P, 512], bf16, tag=f"z{dblk}", name=f"z{dblk}")
                    nc.vector.tensor_copy(z, pz)
                    z_sb.append(z)

                # output projection + deferred normalization + bias.
                # nch inner next to the shared z lhsT so consecutive matmuls
                # reuse the stationary operand (half the LDWEIGHTS).
                for s in range(4):
                    r0 = ch * 512 + s * P
                    pys = [psy.tile([P, 512], f32, tag=f"y{j}", name=f"py{j}")
                           for j in range(2)]
                    for dblk in range(V_F // P):
                        for nch in range(2):
                            nc.tensor.matmul(
                                pys[nch],
                                z_sb[dblk][:, s * P:(s + 1) * P],
                                wo_sb[dblk][:, nch * 512:(nch + 1) * 512],
                                start=(dblk == 0), stop=(dblk == V_F // P - 1),
                            )
                    for nch in range(2):
                        ysb = ysp.tile([P, 512], f32, tag="ysb")
                        nc.vector.scalar_tensor_tensor(
                            ysb, pys[nch], rc[s], bob[:, nch * 512:(nch + 1) * 512],
                            op0=ALU.mult, op1=ALU.add,
                        )
                        nc.sync.dma_start(
                            y_d[r0:r0 + P, nch * 512:(nch + 1) * 512], ysb
                        )

    with tile.TileContext(nc) as tc:
        for r in range(reps):
            if r:
                tc.strict_bb_all_engine_barrier()
            with ExitStack() as ctx:
                _emit_body(tc, ctx)

    nc.compile()
    return nc


def get_nc(reps=1):
    key = f"nc{reps}"
    if key not in _NC_CACHE:
        _NC_CACHE[key] = _build_program(reps)
    return _NC_CACHE[key]


def prepare_in_maps(inputs):
    bf = ml_dtypes.bfloat16
    f32 = np.float32

    q_real = np.asarray(inputs["q_real"], f32)
    q_imag = np.asarray(inputs["q_imag"], f32)
    k_real = np.asarray(inputs["k_real"], f32)
    k_imag = np.asarray(inputs["k_imag"], f32)
    v_real = np.asarray(inputs["v_real"], f32)
    v_imag = np.asarray(inputs["v_imag"], f32)
    pad_mask = np.asarray(inputs["pad_mask"]).astype(bool)
    W_qkv = np.asarray(inputs["W_qkv"], f32)
    b_qkv = np.asarray(inputs["b_qkv"], f32)
    W_out = np.asarray(inputs["W_out"], f32)
    b_out = np.asarray(inputs["b_out"], f32)

    sel_qk = np.r_[0:D, 2 * D:3 * D]          # q_real + k_real output blocks
    wqkT = np.ascontiguousarray(W_qkv[sel_qk, :].T.astype(bf))
    wvT = np.ascontiguousarray(W_qkv[4 * D:6 * D, :].T.astype(bf))
    woT = np.ascontiguousarray(W_out.T.astype(bf))
    bqk = np.ascontiguousarray(b_qkv[sel_qk].reshape(QK_F // P, P).T.astype(f32))
    bvb = np.ascontiguousarray(np.broadcast_to(b_qkv[4 * D:6 * D], (P, V_F)).astype(f32))
    bob = np.ascontiguousarray(np.broadcast_to(b_out, (P, V_F)).astype(f32))

    x = np.concatenate([q_real, q_imag, k_real, k_imag, v_real, v_imag], axis=-1)

    in_maps = []
    for c in range(NCORES):
        b, h = divmod(c, 2)
        xT = np.ascontiguousarray(x[b][h * HALF:(h + 1) * HALF].T.astype(bf))
        # -ln(4): scales the unnormalized exp weights into comfortable fp8e4
        # range; cancels exactly in the softmax normalization.
        mbias = np.where(pad_mask[b], f32(NEG), f32(-1.3862944))
        mbt = np.ascontiguousarray(mbias.reshape(L // P, P).T.astype(f32))
        in_maps.append({
            "xT": xT, "wqkT": wqkT, "wvT": wvT, "woT": woT,
            "maskb": mbt, "bqk": bqk, "bvb": bvb, "bob": bob,
        })
    return in_maps


def assemble_outputs(results):
    out_real = np.empty((B, L, D), np.float32)
    out_imag = np.empty((B, L, D), np.float32)
    for c in range(NCORES):
        y = np.asarray(results[c]["y"], np.float32)
        b, h = divmod(c, 2)
        out_real[b, h * HALF:(h + 1) * HALF] = y[:, :D]
        out_imag[b, h * HALF:(h + 1) * HALF] = y[:, D:]
    return out_real, out_imag


def _make_executor(reps=1):
    """One jitted SPMD callable per process (mirrors bass2jax.run_bass_via_pjrt
    but is built once and reused, so repeated runs don't recompile)."""
    import jax
    from concourse import bass2jax, mybir

    try:
        jax.config.update("jax_compilation_cache_dir", "/tmp/jax_neff_cache")
        jax.config.update("jax_persistent_cache_min_compile_time_secs", 5.0)
    except Exception:
        pass

    nc = get_nc(reps)
    bass2jax.install_neuronx_cc_hook()
    partition_name = nc.partition_id_tensor.name if nc.partition_id_tensor else None

    in_names, out_names, out_avals, zero_outs = [], [], [], []
    for alloc in nc.m.functions[0].allocations:
        if not isinstance(alloc, mybir.MemoryLocationSet):
            continue
        name = alloc.memorylocations[0].name
        if alloc.kind == "ExternalInput":
            if name != partition_name:
                in_names.append(name)
        elif alloc.kind == "ExternalOutput":
            out_names.append(name)
            shape = tuple(alloc.tensor_shape)
            dtype = mybir.dt.np(alloc.dtype)
            out_avals.append(jax.core.ShapedArray(shape, dtype))
            zero_outs.append((shape, dtype))
    n_params = len(in_names)
    n_outs = len(out_avals)
    all_in_names = list(in_names) + list(out_names)
    if partition_name is not None:
        all_in_names.append(partition_name)

    def _body(*args):
        operands = list(args)
        if partition_name is not None:
            operands.append(bass2jax.partition_id_tensor())
        outs = bass2jax._bass_exec_p.bind(
            *operands,
            out_avals=tuple(out_avals),
            in_names=tuple(all_in_names),
            out_names=tuple(out_names),
            lowering_input_output_aliases=(),
            sim_require_finite=True,
            sim_require_nnan=True,
            nc=nc,
        )
        return tuple(outs)

    devices = jax.devices()[:NCORES]
    assert len(devices) == NCORES
    mesh = bass2jax.Mesh(np.asarray(devices), ("core",))
    in_specs = (bass2jax.PartitionSpec("core"),) * (n_params + n_outs)
    out_specs = (bass2jax.PartitionSpec("core"),) * n_outs
    donate = tuple(range(n_params, n_params + n_outs))
    sharded = jax.jit(
        bass2jax.shard_map(
            _body, mesh=mesh, in_specs=in_specs,
            out_specs=out_specs, check_rep=False,
        ),
        donate_argnums=donate,
        keep_unused=True,
    )
    return {
        "sharded": sharded,
        "mesh": mesh,
        "in_names": in_names,
        "out_names": out_names,
        "out_avals": out_avals,
        "zero_outs": zero_outs,
    }


def get_executor(reps=1):
    key = f"exec{reps}"
    if key not in _NC_CACHE:
        _NC_CACHE[key] = _make_executor(reps)
    return _NC_CACHE[key]


def concat_inputs(in_maps, ex):
    return [
        np.concatenate([np.asarray(in_maps[c][n]) for c in range(NCORES)], axis=0)
        for n in ex["in_names"]
    ]


def make_zero_outs(ex):
    return [
        np.zeros((NCORES * s[0], *s[1:]), d) for (s, d) in ex["zero_outs"]
    ]


def execute(concat_in, ex):
    out_arrs = ex["sharded"](*concat_in, *make_zero_outs(ex))
    results = [
        {
            name: np.asarray(out_arrs[i]).reshape(
                NCORES, *ex["out_avals"][i].shape
            )[c]
            for i, name in enumerate(ex["out_names"])
        }
        for c in range(NCORES)
    ]
    return results


def run(inputs, trace=False):
    from concourse.bass_utils import run_bass_kernel_spmd

    nc = get_nc()
    in_maps = prepare_in_maps(inputs)
    return run_bass_kernel_spmd(
        nc, in_maps, core_ids=list(range(NCORES)), trace=trace
    )


def kernel(**inputs):
    ex = get_executor()
    in_maps = prepare_in_maps(inputs)
    results = execute(concat_inputs(in_maps, ex), ex)
    return assemble_outputs(results)



# revision 27
# speedup vs baseline: 1.0265x; 1.0265x over previous
"""Trainium2 Bass kernel for BasicQuantumAttention.

Contract: kernel(**inputs) takes the FULL (unsharded) numpy inputs of the
reference problem (B=4, L=2048, D=512) and returns the full output
(out_real, out_imag), each [B, L, D] float32.

Sharding: 8 NeuronCores; core c handles batch b=c//2, query half h=c%2
(1024 queries). Each core computes the fused QKV projection only for its
own 1024 rows; the key/value projections are then exchanged within the
core pair via an in-kernel pairwise AllGather, so no projection work is
duplicated. Key order is global (rows 0:2047 of the batch) on both cores
of a pair, which keeps the SPMD program identical on every core.

Layouts (all matmuls bf16, f32 PSUM accumulation):
  - x is passed transposed per core: xT [6D=3072, own 1024 rows].
  - q,k projections are computed weight-stationary into ^T layout
    [feat, row]; v is computed into row-major [row, feat] layout.
  - Only the qkv output blocks the reference actually uses are computed
    (q_real, k_real, v_real, v_imag) -- 2/3 of the fused projection.
  - scores^T [key, query] = (k^T tile).T @ q^T; the per-key padding mask
    and the 1/sqrt(D) scale fold into the ACT Exp (bias/scale).
  - attn^T tiles directly feed Z^T accumulation; an M=1 ones-matmul
    produces the softmax row sums; normalization is deferred to after
    the output projection (diag scaling commutes with the right-matmul),
    and b_out is added in the same fused DVE op.
"""

import numpy as np
import ml_dtypes

B, L, D = 4, 2048, 512
P = 128
IN_F = 6 * D          # 3072 input features of the fused projection
QK_F = 2 * D          # selected output features: q_real block + k_real block
V_F = 2 * D           # selected output features: v_real block + v_imag block
KT = IN_F // P        # 24 contraction tiles
NCORES = 8
HALF = L // 2         # 1024 rows owned per core
SCALE = float(D) ** -0.5
NEG = -30000.0        # additive key mask (exp underflows to exactly 0)
GROUPS = [[0, 1], [2, 3], [4, 5], [6, 7]]
NSEND = 12            # blocks of [128, HALF] sent to the pair: 4 k^T + 8 v

_NC_CACHE = {}


def _build_program(reps=1):
    import os
    import concourse.bass as bass
    import concourse.bacc as bacc
    import concourse.mybir as mybir
    import concourse.tile as tile
    from contextlib import ExitStack

    # Timing-ablation switch: skip the pair exchanges and read back own data
    # (incorrect results, identical instruction shape otherwise).
    NO_COLL = bool(os.environ.get("KERNEL_NO_COLL"))

    f32 = mybir.dt.float32
    bf16 = mybir.dt.bfloat16
    f8 = mybir.dt.float8e4
    DR = mybir.MatmulPerfMode.DoubleRow
    AF = mybir.ActivationFunctionType
    ALU = mybir.AluOpType
    PSUM = bass.MemorySpace.PSUM

    nc = bacc.Bacc(
        "TRN2",
        debug=False,
        enable_asserts=False,
        target_bir_lowering=False,
        num_devices=NCORES,
    )

    xT_d = nc.dram_tensor("xT", [IN_F, HALF], bf16, kind="ExternalInput").ap()
    wqk_d = nc.dram_tensor("wqkT", [IN_F, QK_F], bf16, kind="ExternalInput").ap()
    wv_d = nc.dram_tensor("wvT", [IN_F, V_F], bf16, kind="ExternalInput").ap()
    wo_d = nc.dram_tensor("woT", [V_F, V_F], bf16, kind="ExternalInput").ap()
    mb_d = nc.dram_tensor("maskb", [P, L // P], f32, kind="ExternalInput").ap()
    bqk_d = nc.dram_tensor("bqk", [P, QK_F // P], f32, kind="ExternalInput").ap()
    bvb_d = nc.dram_tensor("bvb", [P, V_F], f32, kind="ExternalInput").ap()
    bob_d = nc.dram_tensor("bob", [P, V_F], f32, kind="ExternalInput").ap()
    y_d = nc.dram_tensor("y", [HALF, V_F], f32, kind="ExternalOutput").ap()

    k_send = nc.dram_tensor("k_send", [4 * P, HALF], bf16).ap()
    k_recv = nc.dram_tensor("k_recv", [8 * P, HALF], bf16).ap()
    v_send0 = nc.dram_tensor("v_send0", [8 * P, 512], bf16).ap()
    v_recv0 = nc.dram_tensor("v_recv0", [16 * P, 512], bf16).ap()
    v_send1 = nc.dram_tensor("v_send1", [8 * P, 512], bf16).ap()
    v_recv1 = nc.dram_tensor("v_recv1", [16 * P, 512], bf16).ap()

    xT_r = xT_d.rearrange("(t p) n -> t p n", p=P)
    wqk_r = wqk_d.rearrange("(t p) n -> t p n", p=P)
    wv_r = wv_d.rearrange("(t p) n -> t p n", p=P)
    wo_r = wo_d.rearrange("(t p) n -> t p n", p=P)
    ksend_r = k_send.rearrange("(i p) n -> i p n", p=P)
    krecv_r = k_recv.rearrange("(i p) n -> i p n", p=P)
    vsend_r = [v.rearrange("(i p) n -> i p n", p=P) for v in (v_send0, v_send1)]
    vrecv_r = [v.rearrange("(i p) n -> i p n", p=P) for v in (v_recv0, v_recv1)]

    def _emit_body(tc, ctx):
        const = ctx.enter_context(tc.tile_pool(name="const", bufs=1))
        persist = ctx.enter_context(tc.tile_pool(name="persist", bufs=1))

        mb = const.tile([P, L // P], f32, tag="mb")
        nc.sync.dma_start(mb, mb_d)
        bqk = const.tile([P, QK_F // P], f32, tag="bqk")
        nc.sync.dma_start(bqk, bqk_d)
        ones_c = const.tile([P, 1], f32, tag="ones_c")
        nc.vector.memset(ones_c, 1.0)
        ident1 = const.tile([1, 1], f32, tag="ident1")
        nc.vector.memset(ident1, 1.0)

        # Free-dim biases come pre-broadcast from the host (loaded after the
        # critical projection streams are queued).
        bvb = persist.tile([P, V_F], f32, tag="bvb")
        bob = persist.tile([P, V_F], f32, tag="bob")

        # Persistent attention operands + output-projection weights.
        q_sb = [persist.tile([P, HALF], bf16, tag=f"q{m}", name=f"q{m}") for m in range(4)]
        k_sb = [persist.tile([P, L], bf16, tag=f"k{m}", name=f"k{m}") for m in range(4)]
        v_sb = [persist.tile([P, V_F], bf16, tag=f"v{rb}", name=f"v{rb}") for rb in range(L // P)]
        wo_sb = [persist.tile([P, V_F], bf16, tag=f"wo{t}", name=f"wo{t}") for t in range(V_F // P)]

        # Staging tiles for the pair exchange (own k^T blocks + own v rows).
        kst = [persist.tile([P, HALF], bf16, tag=f"kst{m}", name=f"kst{m}") for m in range(4)]

        # Phase order: k-proj -> AG(k) -> v-proj (c0 -> AG(v0), c1 -> AG(v1))
        # -> q-proj -> attention.  Each exchange gets 40-120us of projection
        # matmuls as cover; readback DMAs are issued late so a pending
        # collective can never head-of-line-block a weight stream.
        with (
            tc.tile_pool(name="xp", bufs=1) as xp,
            tc.tile_pool(name="ws", bufs=9) as ws,
            tc.tile_pool(name="vstp", bufs=2) as vstp,
        ):
            x = []
            wk = []
            for k in range(KT):
                xt = xp.tile([P, HALF], bf16, tag=f"x{k}")
                nc.sync.dma_start(xt, xT_r[k])
                x.append(xt)
                wkt = ws.tile([P, 512], bf16, tag="wst", name=f"wk{k}")
                nc.sync.dma_start(wkt, wqk_r[k, :, 512:])
                wk.append(wkt)
            nc.sync.dma_start(bvb, bvb_d)

            # ---- k_real projection (^T layout), single pass, 8 PSUM banks.
            # PSUM accumulation is order-independent, so the last TAIL
            # k-steps are emitted chain-major: chain m finishes TAIL*(3-m)
            # k-steps early and its ACT drain + send DMA overlap the
            # remaining matmuls instead of serializing at the phase edge.
            TAIL = 3
            with tc.tile_pool(name="psk", bufs=1, space=PSUM) as psk:
                pks = [psk.tile([P, HALF], f32, tag=f"pk{m}", name=f"pk{m}")
                       for m in range(4)]
                for k in range(KT - TAIL):
                    for m in range(4):
                        for c in range(2):
                            nc.tensor.matmul(
                                pks[m][:, c * 512:(c + 1) * 512],
                                wk[k][:, m * P:(m + 1) * P],
                                x[k][:, c * 512:(c + 1) * 512],
                                start=(k == 0), stop=False,
                            )
                for m in range(4):
                    for k in range(KT - TAIL, KT):
                        for c in range(2):
                            nc.tensor.matmul(
                                pks[m][:, c * 512:(c + 1) * 512],
                                wk[k][:, m * P:(m + 1) * P],
                                x[k][:, c * 512:(c + 1) * 512],
                                start=False, stop=(k == KT - 1),
                            )
                    nc.scalar.activation(kst[m], pks[m], AF.Identity, bias=bqk[:, 4 + m:5 + m])
                    nc.sync.dma_start(ksend_r[m], kst[m])

            # Pairwise exchange of the k^T blocks (ranks are [even, odd] =
            # [global first half, global second half] for both pair members);
            # hidden behind the v and q projections that follow.
            if not NO_COLL:
                nc.gpsimd.collective_compute(
                    "AllGather", mybir.AluOpType.bypass,
                    replica_groups=GROUPS,
                    ins=[k_send.opt()], outs=[k_recv.opt()],
                )

            # ---- v_real|v_imag projection, row-major, c-outer (wv loaded
            # once); each 512-feature half is exchanged as soon as it's done.
            for c in range(2):
                with tc.tile_pool(name="psv", bufs=1, space=PSUM) as psv:
                    pv = [psv.tile([P, 512], f32, tag=f"pv{j}", name=f"pv{j}")
                          for j in range(8)]
                    wv_sl = []
                    for k in range(KT):
                        wsl = ws.tile([P, 512], bf16, tag="wst", name=f"wv{c}_{k}")
                        nc.sync.dma_start(wsl, wv_r[k, :, c * 512:(c + 1) * 512])
                        wv_sl.append(wsl)
                        if k >= KT - TAIL:
                            continue
                        for j in range(8):
                            nc.tensor.matmul(
                                pv[j], x[k][:, j * P:(j + 1) * P], wsl,
                                start=(k == 0), stop=False,
                            )
                    for j in range(8):
                        for k in range(KT - TAIL, KT):
                            nc.tensor.matmul(
                                pv[j], x[k][:, j * P:(j + 1) * P], wv_sl[k],
                                start=False, stop=(k == KT - 1),
                            )
                        vs = vstp.tile([P, 512], bf16, tag=f"vst{j}", name=f"vst{c}_{j}")
                        nc.vector.tensor_tensor(
                            vs, pv[j], bvb[:, c * 512:(c + 1) * 512], op=ALU.add,
                        )
                        nc.sync.dma_start(vsend_r[c][j], vs)
                if not NO_COLL:
                    nc.gpsimd.collective_compute(
                        "AllGather", mybir.AluOpType.bypass,
                        replica_groups=GROUPS,
                        ins=[(v_send0 if c == 0 else v_send1).opt()],
                        outs=[(v_recv0 if c == 0 else v_recv1).opt()],
                    )
                if c == 1:
                    # v1 readback on the gpsimd issue queue: it is busy until
                    # AG(v1) completes there anyway, so the readback starts
                    # the moment its data lands, blocking nothing.
                    for t in range(L // P):
                        nc.gpsimd.dma_start(
                            v_sb[t][:, 512:1024],
                            vsend_r[1][t % 8] if NO_COLL else vrecv_r[1][t],
                        )
                if c == 0:
                    # k readback on the gpsimd queue right after AG(v0)'s
                    # issue: it delays neither AG(v0) (already issued) nor
                    # the k_sb deadline (scores need it ~50us later), and a
                    # long AG(k) can only delay these, never a weight stream.
                    for hh in range(2):
                        for m in range(4):
                            nc.gpsimd.dma_start(
                                k_sb[m][:, hh * HALF:(hh + 1) * HALF],
                                ksend_r[m] if NO_COLL else krecv_r[hh * 4 + m],
                            )
                    nc.sync.dma_start(bob, bob_d)
                    for t in range(V_F // P):
                        nc.sync.dma_start(wo_sb[t], wo_r[t])

            # ---- q_real projection (^T layout), single pass, 8 PSUM banks;
            # overlaps both exchanges.
            with tc.tile_pool(name="psq", bufs=1, space=PSUM) as psq:
                pq = [psq.tile([P, HALF], f32, tag=f"pq{m}", name=f"pq{m}")
                      for m in range(4)]
                wq_sl = []
                for k in range(KT):
                    wsl = ws.tile([P, 512], bf16, tag="wst", name=f"wq{k}")
                    nc.sync.dma_start(wsl, wqk_r[k, :, 0:512])
                    wq_sl.append(wsl)
                    if k >= KT - TAIL:
                        continue
                    for m in range(4):
                        for c in range(2):
                            nc.tensor.matmul(
                                pq[m][:, c * 512:(c + 1) * 512],
                                wsl[:, m * P:(m + 1) * P],
                                x[k][:, c * 512:(c + 1) * 512],
                                start=(k == 0), stop=False,
                            )
                # Staggered tails; the ch0 half of each bias ACT is emitted
                # first so the first scores chain only waits ~0.4us.
                for m in range(4):
                    for k in range(KT - TAIL, KT):
                        for c in range(2):
                            nc.tensor.matmul(
                                pq[m][:, c * 512:(c + 1) * 512],
                                wq_sl[k][:, m * P:(m + 1) * P],
                                x[k][:, c * 512:(c + 1) * 512],
                                start=False, stop=(k == KT - 1),
                            )
                    nc.scalar.activation(q_sb[m][:, 0:512], pq[m][:, 0:512],
                                         AF.Identity, bias=bqk[:, m:m + 1])
                for m in range(4):
                    nc.scalar.activation(q_sb[m][:, 512:HALF], pq[m][:, 512:HALF],
                                         AF.Identity, bias=bqk[:, m:m + 1])

            # v0 readback on SP after every weight stream is queued: AG(v0)
            # completed during the v c1 pass, so this drains immediately and
            # only the late y-output DMAs share its completion lanes.
            for t in range(L // P):
                nc.sync.dma_start(
                    v_sb[t][:, 0:512],
                    vsend_r[0][t % 8] if NO_COLL else vrecv_r[0][t],
                )

        # --------------------------- attention ---------------------------
        with (
            tc.tile_pool(name="at", bufs=2) as atp,
            tc.tile_pool(name="zs", bufs=2) as zsp,
            tc.tile_pool(name="ys", bufs=2) as ysp,
            tc.tile_pool(name="sm", bufs=2) as smp,
            tc.tile_pool(name="accs", bufs=2) as accp,
            tc.tile_pool(name="pssc", bufs=2, space=PSUM) as pssc,
            tc.tile_pool(name="psz", bufs=2, space=PSUM) as psz,
            tc.tile_pool(name="pssum", bufs=1, space=PSUM) as pssum,
            tc.tile_pool(name="pstp", bufs=1, space=PSUM) as pstp,
            tc.tile_pool(name="psy", bufs=1, space=PSUM) as psy,
        ):
            # Both chunks' scores+exp are emitted before any Z work: 28us of
            # guaranteed PE work that covers a late AG(v1) regardless of how
            # slow the exchange chain runs.
            at_all = []
            for ch in range(2):
                qs = slice(ch * 512, (ch + 1) * 512)
                at = []
                for t in range(L // P):
                    ps = pssc.tile([P, 512], f32, tag="sc")
                    for d in range(4):
                        nc.tensor.matmul(
                            ps, k_sb[d][:, t * P:(t + 1) * P], q_sb[d][:, qs],
                            start=(d == 0), stop=(d == 3),
                        )
                    a = atp.tile([P, 512], bf16, tag=f"at{t}")
                    nc.scalar.activation(
                        a, ps, AF.Exp, bias=mb[:, t:t + 1], scale=SCALE
                    )
                    at.append(a)
                at_all.append(at)

            for ch in range(2):
                at = at_all[ch]

                # softmax row-sums: accumulate the at tiles on the (idle) DVE
                # and partition-reduce with a single f32 ones-matmul instead
                # of 16 PE matmuls.
                acc = accp.tile([P, 512], f32, tag="acc")
                nc.vector.tensor_tensor(acc, at[0], at[1], op=ALU.add)
                for t in range(2, L // P):
                    nc.vector.tensor_tensor(acc, acc, at[t], op=ALU.add)
                sp = pssum.tile([1, 512], f32, tag="sum")
                nc.tensor.matmul(sp, ones_c, acc, start=True, stop=True)
                sums = smp.tile([1, 512], f32, tag="sums")
                nc.vector.tensor_copy(sums, sp)
                rc = []
                for s in range(4):
                    tp = pstp.tile([P, 1], f32, tag="tp")
                    nc.tensor.transpose(tp, sums[0:1, s * P:(s + 1) * P], ident1)
                    r = smp.tile([P, 1], f32, tag=f"rc{s}", name=f"rc{s}")
                    nc.vector.reciprocal(r, tp)
                    rc.append(r)

                # Z^T = sum_key v[key, dblk] * attn^T[key, q]
                z_sb = []
                for dblk in range(V_F // P):
                    pz = psz.tile([P, 512], f32, tag="z")
                    for t in range(L // P):
                        nc.tensor.matmul(
                            pz, v_sb[t][:, dblk * P:(dblk + 1) * P], at[t],
                            start=(t == 0), stop=(t == L // P - 1),
                        )
                    z = zsp.tile([P, 512], bf16, tag=f"z{dblk}", name=f"z{dblk}")
                    nc.vector.tensor_copy(z, pz)
                    z_sb.append(z)

                # output projection + deferred normalization + bias.
                # nch inner next to the shared z lhsT so consecutive matmuls
                # reuse the stationary operand (half the LDWEIGHTS).
                for s in range(4):
                    r0 = ch * 512 + s * P
                    pys = [psy.tile([P, 512], f32, tag=f"y{j}", name=f"py{j}")
                           for j in range(2)]
                    for dblk in range(V_F // P):
                        for nch in range(2):
                            nc.tensor.matmul(
                                pys[nch],
                                z_sb[dblk][:, s * P:(s + 1) * P],
                                wo_sb[dblk][:, nch * 512:(nch + 1) * 512],
                                start=(dblk == 0), stop=(dblk == V_F // P - 1),
                            )
                    for nch in range(2):
                        ysb = ysp.tile([P, 512], f32, tag="ysb")
                        nc.vector.scalar_tensor_tensor(
                            ysb, pys[nch], rc[s], bob[:, nch * 512:(nch + 1) * 512],
                            op0=ALU.mult, op1=ALU.add,
                        )
                        nc.sync.dma_start(
                            y_d[r0:r0 + P, nch * 512:(nch + 1) * 512], ysb
                        )

    with tile.TileContext(nc) as tc:
        for r in range(reps):
            if r:
                tc.strict_bb_all_engine_barrier()
            with ExitStack() as ctx:
                _emit_body(tc, ctx)

    nc.compile()
    return nc


def get_nc(reps=1):
    key = f"nc{reps}"
    if key not in _NC_CACHE:
        _NC_CACHE[key] = _build_program(reps)
    return _NC_CACHE[key]


def prepare_in_maps(inputs):
    bf = ml_dtypes.bfloat16
    f32 = np.float32

    q_real = np.asarray(inputs["q_real"], f32)
    q_imag = np.asarray(inputs["q_imag"], f32)
    k_real = np.asarray(inputs["k_real"], f32)
    k_imag = np.asarray(inputs["k_imag"], f32)
    v_real = np.asarray(inputs["v_real"], f32)
    v_imag = np.asarray(inputs["v_imag"], f32)
    pad_mask = np.asarray(inputs["pad_mask"]).astype(bool)
    W_qkv = np.asarray(inputs["W_qkv"], f32)
    b_qkv = np.asarray(inputs["b_qkv"], f32)
    W_out = np.asarray(inputs["W_out"], f32)
    b_out = np.asarray(inputs["b_out"], f32)

    sel_qk = np.r_[0:D, 2 * D:3 * D]          # q_real + k_real output blocks
    wqkT = np.ascontiguousarray(W_qkv[sel_qk, :].T.astype(bf))
    wvT = np.ascontiguousarray(W_qkv[4 * D:6 * D, :].T.astype(bf))
    woT = np.ascontiguousarray(W_out.T.astype(bf))
    bqk = np.ascontiguousarray(b_qkv[sel_qk].reshape(QK_F // P, P).T.astype(f32))
    bvb = np.ascontiguousarray(np.broadcast_to(b_qkv[4 * D:6 * D], (P, V_F)).astype(f32))
    bob = np.ascontiguousarray(np.broadcast_to(b_out, (P, V_F)).astype(f32))

    x = np.concatenate([q_real, q_imag, k_real, k_imag, v_real, v_imag], axis=-1)

    in_maps = []
    for c in range(NCORES):
        b, h = divmod(c, 2)
        xT = np.ascontiguousarray(x[b][h * HALF:(h + 1) * HALF].T.astype(bf))
        # -ln(4): scales the unnormalized exp weights into comfortable fp8e4
        # range; cancels exactly in the softmax normalization.
        mbias = np.where(pad_mask[b], f32(NEG), f32(-1.3862944))
        mbt = np.ascontiguousarray(mbias.reshape(L // P, P).T.astype(f32))
        in_maps.append({
            "xT": xT, "wqkT": wqkT, "wvT": wvT, "woT": woT,
            "maskb": mbt, "bqk": bqk, "bvb": bvb, "bob": bob,
        })
    return in_maps


def assemble_outputs(results):
    out_real = np.empty((B, L, D), np.float32)
    out_imag = np.empty((B, L, D), np.float32)
    for c in range(NCORES):
        y = np.asarray(results[c]["y"], np.float32)
        b, h = divmod(c, 2)
        out_real[b, h * HALF:(h + 1) * HALF] = y[:, :D]
        out_imag[b, h * HALF:(h + 1) * HALF] = y[:, D:]
    return out_real, out_imag


def _make_executor(reps=1):
    """One jitted SPMD callable per process (mirrors bass2jax.run_bass_via_pjrt
    but is built once and reused, so repeated runs don't recompile)."""
    import jax
    from concourse import bass2jax, mybir

    try:
        jax.config.update("jax_compilation_cache_dir", "/tmp/jax_neff_cache")
        jax.config.update("jax_persistent_cache_min_compile_time_secs", 5.0)
    except Exception:
        pass

    nc = get_nc(reps)
    bass2jax.install_neuronx_cc_hook()
    partition_name = nc.partition_id_tensor.name if nc.partition_id_tensor else None

    in_names, out_names, out_avals, zero_outs = [], [], [], []
    for alloc in nc.m.functions[0].allocations:
        if not isinstance(alloc, mybir.MemoryLocationSet):
            continue
        name = alloc.memorylocations[0].name
        if alloc.kind == "ExternalInput":
            if name != partition_name:
                in_names.append(name)
        elif alloc.kind == "ExternalOutput":
            out_names.append(name)
            shape = tuple(alloc.tensor_shape)
            dtype = mybir.dt.np(alloc.dtype)
            out_avals.append(jax.core.ShapedArray(shape, dtype))
            zero_outs.append((shape, dtype))
    n_params = len(in_names)
    n_outs = len(out_avals)
    all_in_names = list(in_names) + list(out_names)
    if partition_name is not None:
        all_in_names.append(partition_name)

    def _body(*args):
        operands = list(args)
        if partition_name is not None:
            operands.append(bass2jax.partition_id_tensor())
        outs = bass2jax._bass_exec_p.bind(
            *operands,
            out_avals=tuple(out_avals),
            in_names=tuple(all_in_names),
            out_names=tuple(out_names),
            lowering_input_output_aliases=(),
            sim_require_finite=True,
            sim_require_nnan=True,
            nc=nc,
        )
        return tuple(outs)

    devices = jax.devices()[:NCORES]
    assert len(devices) == NCORES
    mesh = bass2jax.Mesh(np.asarray(devices), ("core",))
    in_specs = (bass2jax.PartitionSpec("core"),) * (n_params + n_outs)
    out_specs = (bass2jax.PartitionSpec("core"),) * n_outs
    donate = tuple(range(n_params, n_params + n_outs))
    sharded = jax.jit(
        bass2jax.shard_map(
            _body, mesh=mesh, in_specs=in_specs,
            out_specs=out_specs, check_rep=False,
        ),
        donate_argnums=donate,
        keep_unused=True,
    )
    return {
        "sharded": sharded,
        "mesh": mesh,
        "in_names": in_names,
        "out_names": out_names,
        "out_avals": out_avals,
        "zero_outs": zero_outs,
    }


def get_executor(reps=1):
    key = f"exec{reps}"
    if key not in _NC_CACHE:
        _NC_CACHE[key] = _make_executor(reps)
    return _NC_CACHE[key]


def concat_inputs(in_maps, ex):
    return [
        np.concatenate([np.asarray(in_maps[c][n]) for c in range(NCORES)], axis=0)
        for n in ex["in_names"]
    ]


def make_zero_outs(ex):
    return [
        np.zeros((NCORES * s[0], *s[1:]), d) for (s, d) in ex["zero_outs"]
    ]


def execute(concat_in, ex):
    out_arrs = ex["sharded"](*concat_in, *make_zero_outs(ex))
    results = [
        {
            name: np.asarray(out_arrs[i]).reshape(
                NCORES, *ex["out_avals"][i].shape
            )[c]
            for i, name in enumerate(ex["out_names"])
        }
        for c in range(NCORES)
    ]
    return results


def run(inputs, trace=False):
    from concourse.bass_utils import run_bass_kernel_spmd

    nc = get_nc()
    in_maps = prepare_in_maps(inputs)
    return run_bass_kernel_spmd(
        nc, in_maps, core_ids=list(range(NCORES)), trace=trace
    )


def kernel(**inputs):
    ex = get_executor()
    in_maps = prepare_in_maps(inputs)
    results = execute(concat_inputs(in_maps, ex), ex)
    return assemble_outputs(results)



# revision 28
# speedup vs baseline: 1.1300x; 1.1008x over previous
"""Trainium2 Bass kernel for BasicQuantumAttention.

Contract: kernel(**inputs) takes the FULL (unsharded) numpy inputs of the
reference problem (B=4, L=2048, D=512) and returns the full output
(out_real, out_imag), each [B, L, D] float32.

Sharding: 8 NeuronCores; core c handles batch b=c//2, query half h=c%2
(1024 queries). Each core computes the fused QKV projection only for its
own 1024 rows; the key/value projections are then exchanged within the
core pair via an in-kernel pairwise AllGather, so no projection work is
duplicated. Key order is global (rows 0:2047 of the batch) on both cores
of a pair, which keeps the SPMD program identical on every core.

Layouts (all matmuls bf16, f32 PSUM accumulation):
  - x is passed transposed per core: xT [6D=3072, own 1024 rows].
  - q,k projections are computed weight-stationary into ^T layout
    [feat, row]; v is computed into row-major [row, feat] layout.
  - Only the qkv output blocks the reference actually uses are computed
    (q_real, k_real, v_real, v_imag) -- 2/3 of the fused projection.
  - scores^T [key, query] = (k^T tile).T @ q^T; the per-key padding mask
    and the 1/sqrt(D) scale fold into the ACT Exp (bias/scale).
  - attn^T tiles directly feed Z^T accumulation; an M=1 ones-matmul
    produces the softmax row sums; normalization is deferred to after
    the output projection (diag scaling commutes with the right-matmul),
    and b_out is added in the same fused DVE op.
"""

import numpy as np
import ml_dtypes

B, L, D = 4, 2048, 512
P = 128
IN_F = 6 * D          # 3072 input features of the fused projection
QK_F = 2 * D          # selected output features: q_real block + k_real block
V_F = 2 * D           # selected output features: v_real block + v_imag block
KT = IN_F // P        # 24 contraction tiles
NCORES = 8
HALF = L // 2         # 1024 rows owned per core
SCALE = float(D) ** -0.5
NEG = -30000.0        # additive key mask (exp underflows to exactly 0)
GROUPS = [[0, 1], [2, 3], [4, 5], [6, 7]]
NSEND = 12            # blocks of [128, HALF] sent to the pair: 4 k^T + 8 v

_NC_CACHE = {}


def _build_program(reps=1):
    import os
    import concourse.bass as bass
    import concourse.bacc as bacc
    import concourse.mybir as mybir
    import concourse.tile as tile
    from contextlib import ExitStack

    # Timing-ablation switch: skip the pair exchanges and read back own data
    # (incorrect results, identical instruction shape otherwise).
    NO_COLL = bool(os.environ.get("KERNEL_NO_COLL"))

    f32 = mybir.dt.float32
    bf16 = mybir.dt.bfloat16
    f8 = mybir.dt.float8e4
    DR = mybir.MatmulPerfMode.DoubleRow
    AF = mybir.ActivationFunctionType
    ALU = mybir.AluOpType
    PSUM = bass.MemorySpace.PSUM

    nc = bacc.Bacc(
        "TRN2",
        debug=False,
        enable_asserts=False,
        target_bir_lowering=False,
        num_devices=NCORES,
    )

    xT_d = nc.dram_tensor("xT", [IN_F, HALF], bf16, kind="ExternalInput").ap()
    wqk_d = nc.dram_tensor("wqkT", [IN_F, QK_F], bf16, kind="ExternalInput").ap()
    wv_d = nc.dram_tensor("wvT", [IN_F, V_F], bf16, kind="ExternalInput").ap()
    wo_d = nc.dram_tensor("woT", [V_F, V_F], bf16, kind="ExternalInput").ap()
    mb_d = nc.dram_tensor("maskb", [P, L // P], f32, kind="ExternalInput").ap()
    bqk_d = nc.dram_tensor("bqk", [P, QK_F // P], f32, kind="ExternalInput").ap()
    bvb_d = nc.dram_tensor("bvb", [P, V_F], f32, kind="ExternalInput").ap()
    bob_d = nc.dram_tensor("bob", [P, V_F], f32, kind="ExternalInput").ap()
    y_d = nc.dram_tensor("y", [HALF, V_F], f32, kind="ExternalOutput").ap()

    k_send = nc.dram_tensor("k_send", [4 * P, HALF], bf16).ap()
    k_recv = nc.dram_tensor("k_recv", [8 * P, HALF], bf16).ap()
    v_send0 = nc.dram_tensor("v_send0", [8 * P, 512], bf16).ap()
    v_recv0 = nc.dram_tensor("v_recv0", [16 * P, 512], bf16).ap()
    v_send1 = nc.dram_tensor("v_send1", [8 * P, 512], bf16).ap()
    v_recv1 = nc.dram_tensor("v_recv1", [16 * P, 512], bf16).ap()

    xT_r = xT_d.rearrange("(t p) n -> t p n", p=P)
    wqk_r = wqk_d.rearrange("(t p) n -> t p n", p=P)
    wv_r = wv_d.rearrange("(t p) n -> t p n", p=P)
    wo_r = wo_d.rearrange("(t p) n -> t p n", p=P)
    ksend_r = k_send.rearrange("(i p) n -> i p n", p=P)
    krecv_r = k_recv.rearrange("(i p) n -> i p n", p=P)
    vsend_r = [v.rearrange("(i p) n -> i p n", p=P) for v in (v_send0, v_send1)]
    vrecv_r = [v.rearrange("(i p) n -> i p n", p=P) for v in (v_recv0, v_recv1)]

    def _emit_body(tc, ctx):
        const = ctx.enter_context(tc.tile_pool(name="const", bufs=1))
        persist = ctx.enter_context(tc.tile_pool(name="persist", bufs=1))

        mb = const.tile([P, L // P], f32, tag="mb")
        nc.sync.dma_start(mb, mb_d)
        bqk = const.tile([P, QK_F // P], f32, tag="bqk")
        nc.sync.dma_start(bqk, bqk_d)
        ones_c = const.tile([P, 1], f32, tag="ones_c")
        nc.vector.memset(ones_c, 1.0)
        ident1 = const.tile([1, 1], f32, tag="ident1")
        nc.vector.memset(ident1, 1.0)

        # Free-dim biases come pre-broadcast from the host (loaded after the
        # critical projection streams are queued).
        bvb = persist.tile([P, V_F], f32, tag="bvb")
        bob = persist.tile([P, V_F], f32, tag="bob")

        # Persistent attention operands + output-projection weights.
        q_sb = [persist.tile([P, HALF], bf16, tag=f"q{m}", name=f"q{m}") for m in range(4)]
        k_sb = [persist.tile([P, L], bf16, tag=f"k{m}", name=f"k{m}") for m in range(4)]
        v_sb = [persist.tile([P, V_F], bf16, tag=f"v{rb}", name=f"v{rb}") for rb in range(L // P)]
        wo_sb = [persist.tile([P, V_F], bf16, tag=f"wo{t}", name=f"wo{t}") for t in range(V_F // P)]

        # Staging tiles for the pair exchange (own k^T blocks + own v rows).
        kst = [persist.tile([P, HALF], bf16, tag=f"kst{m}", name=f"kst{m}") for m in range(4)]

        # Phase order: k-proj -> AG(k) -> v-proj (c0 -> AG(v0), c1 -> AG(v1))
        # -> q-proj -> attention.  Each exchange gets 40-120us of projection
        # matmuls as cover; readback DMAs are issued late so a pending
        # collective can never head-of-line-block a weight stream.
        with (
            tc.tile_pool(name="xp", bufs=1) as xp,
            tc.tile_pool(name="ws", bufs=9) as ws,
            tc.tile_pool(name="vstp", bufs=2) as vstp,
        ):
            x = []
            wk = []
            for k in range(KT):
                xt = xp.tile([P, HALF], bf16, tag=f"x{k}")
                nc.sync.dma_start(xt, xT_r[k])
                x.append(xt)
                wkt = ws.tile([P, 512], bf16, tag="wst", name=f"wk{k}")
                nc.sync.dma_start(wkt, wqk_r[k, :, 512:])
                wk.append(wkt)
            nc.sync.dma_start(bvb, bvb_d)

            # ---- k_real projection (^T layout), single pass, 8 PSUM banks.
            # PSUM accumulation is order-independent, so the last TAIL
            # k-steps are emitted chain-major: chain m finishes TAIL*(3-m)
            # k-steps early and its ACT drain + send DMA overlap the
            # remaining matmuls instead of serializing at the phase edge.
            TAIL = 3
            with tc.tile_pool(name="psk", bufs=1, space=PSUM) as psk:
                pks = [psk.tile([P, HALF], f32, tag=f"pk{m}", name=f"pk{m}")
                       for m in range(4)]
                for k in range(KT - TAIL):
                    for m in range(4):
                        for c in range(2):
                            nc.tensor.matmul(
                                pks[m][:, c * 512:(c + 1) * 512],
                                wk[k][:, m * P:(m + 1) * P],
                                x[k][:, c * 512:(c + 1) * 512],
                                start=(k == 0), stop=False,
                            )
                for m in range(4):
                    for k in range(KT - TAIL, KT):
                        for c in range(2):
                            nc.tensor.matmul(
                                pks[m][:, c * 512:(c + 1) * 512],
                                wk[k][:, m * P:(m + 1) * P],
                                x[k][:, c * 512:(c + 1) * 512],
                                start=False, stop=(k == KT - 1),
                            )
                    nc.scalar.activation(kst[m], pks[m], AF.Identity, bias=bqk[:, 4 + m:5 + m])
                    nc.sync.dma_start(ksend_r[m], kst[m])

            # Pairwise exchange of the k^T blocks (ranks are [even, odd] =
            # [global first half, global second half] for both pair members);
            # hidden behind the v and q projections that follow.
            if not NO_COLL:
                nc.gpsimd.collective_compute(
                    "AllGather", mybir.AluOpType.bypass,
                    replica_groups=GROUPS,
                    ins=[k_send.opt()], outs=[k_recv.opt()],
                )

            # ---- v_real|v_imag projection, row-major, c-outer (wv loaded
            # once); each 512-feature half is exchanged as soon as it's done.
            for c in range(2):
                with tc.tile_pool(name="psv", bufs=1, space=PSUM) as psv:
                    pv = [psv.tile([P, 512], f32, tag=f"pv{j}", name=f"pv{j}")
                          for j in range(8)]
                    wv_sl = []
                    for k in range(KT):
                        wsl = ws.tile([P, 512], bf16, tag="wst", name=f"wv{c}_{k}")
                        nc.sync.dma_start(wsl, wv_r[k, :, c * 512:(c + 1) * 512])
                        wv_sl.append(wsl)
                        if k >= KT - TAIL:
                            continue
                        for j in range(8):
                            nc.tensor.matmul(
                                pv[j], x[k][:, j * P:(j + 1) * P], wsl,
                                start=(k == 0), stop=False,
                            )
                    for j in range(8):
                        for k in range(KT - TAIL, KT):
                            nc.tensor.matmul(
                                pv[j], x[k][:, j * P:(j + 1) * P], wv_sl[k],
                                start=False, stop=(k == KT - 1),
                            )
                        vs = vstp.tile([P, 512], bf16, tag=f"vst{j}", name=f"vst{c}_{j}")
                        nc.vector.tensor_tensor(
                            vs, pv[j], bvb[:, c * 512:(c + 1) * 512], op=ALU.add,
                        )
                        nc.sync.dma_start(vsend_r[c][j], vs)
                if not NO_COLL:
                    nc.gpsimd.collective_compute(
                        "AllGather", mybir.AluOpType.bypass,
                        replica_groups=GROUPS,
                        ins=[(v_send0 if c == 0 else v_send1).opt()],
                        outs=[(v_recv0 if c == 0 else v_recv1).opt()],
                    )
                if c == 1:
                    # v readbacks on the gpsimd issue queue: it is busy with
                    # the exchange chain anyway, so these start the moment
                    # their collective completes and can block nothing else.
                    for t in range(L // P):
                        nc.gpsimd.dma_start(
                            v_sb[t][:, 0:512],
                            vsend_r[0][t % 8] if NO_COLL else vrecv_r[0][t],
                        )
                    for t in range(L // P):
                        nc.gpsimd.dma_start(
                            v_sb[t][:, 512:1024],
                            vsend_r[1][t % 8] if NO_COLL else vrecv_r[1][t],
                        )
                if c == 0:
                    # k readback on the gpsimd queue right after AG(v0)'s
                    # issue: it delays neither AG(v0) (already issued) nor
                    # the k_sb deadline (scores need it ~50us later), and a
                    # long AG(k) can only delay these, never a weight stream.
                    for hh in range(2):
                        for m in range(4):
                            nc.gpsimd.dma_start(
                                k_sb[m][:, hh * HALF:(hh + 1) * HALF],
                                ksend_r[m] if NO_COLL else krecv_r[hh * 4 + m],
                            )
                    nc.sync.dma_start(bob, bob_d)
                    for t in range(V_F // P):
                        nc.sync.dma_start(wo_sb[t], wo_r[t])

            # ---- q_real projection (^T layout), single pass, 8 PSUM banks;
            # overlaps both exchanges.
            with tc.tile_pool(name="psq", bufs=1, space=PSUM) as psq:
                pq = [psq.tile([P, HALF], f32, tag=f"pq{m}", name=f"pq{m}")
                      for m in range(4)]
                wq_sl = []
                for k in range(KT):
                    wsl = ws.tile([P, 512], bf16, tag="wst", name=f"wq{k}")
                    nc.sync.dma_start(wsl, wqk_r[k, :, 0:512])
                    wq_sl.append(wsl)
                    if k >= KT - TAIL:
                        continue
                    for m in range(4):
                        for c in range(2):
                            nc.tensor.matmul(
                                pq[m][:, c * 512:(c + 1) * 512],
                                wsl[:, m * P:(m + 1) * P],
                                x[k][:, c * 512:(c + 1) * 512],
                                start=(k == 0), stop=False,
                            )
                # Staggered tails; the ch0 half of each bias ACT is emitted
                # first so the first scores chain only waits ~0.4us.
                for m in range(4):
                    for k in range(KT - TAIL, KT):
                        for c in range(2):
                            nc.tensor.matmul(
                                pq[m][:, c * 512:(c + 1) * 512],
                                wq_sl[k][:, m * P:(m + 1) * P],
                                x[k][:, c * 512:(c + 1) * 512],
                                start=False, stop=(k == KT - 1),
                            )
                    nc.scalar.activation(q_sb[m][:, 0:512], pq[m][:, 0:512],
                                         AF.Identity, bias=bqk[:, m:m + 1])
                for m in range(4):
                    nc.scalar.activation(q_sb[m][:, 512:HALF], pq[m][:, 512:HALF],
                                         AF.Identity, bias=bqk[:, m:m + 1])


        # --------------------------- attention ---------------------------
        with (
            tc.tile_pool(name="at", bufs=2) as atp,
            tc.tile_pool(name="zs", bufs=2) as zsp,
            tc.tile_pool(name="ys", bufs=2) as ysp,
            tc.tile_pool(name="sm", bufs=2) as smp,
            tc.tile_pool(name="accs", bufs=2) as accp,
            tc.tile_pool(name="pssc", bufs=2, space=PSUM) as pssc,
            tc.tile_pool(name="psz", bufs=2, space=PSUM) as psz,
            tc.tile_pool(name="pssum", bufs=1, space=PSUM) as pssum,
            tc.tile_pool(name="pstp", bufs=1, space=PSUM) as pstp,
            tc.tile_pool(name="psy", bufs=1, space=PSUM) as psy,
        ):
            # Both chunks' scores+exp are emitted before any Z work: 28us of
            # guaranteed PE work that covers a late AG(v1) regardless of how
            # slow the exchange chain runs.
            at_all = []
            for ch in range(2):
                qs = slice(ch * 512, (ch + 1) * 512)
                at = []
                for t in range(L // P):
                    ps = pssc.tile([P, 512], f32, tag="sc")
                    for d in range(4):
                        nc.tensor.matmul(
                            ps, k_sb[d][:, t * P:(t + 1) * P], q_sb[d][:, qs],
                            start=(d == 0), stop=(d == 3),
                        )
                    a = atp.tile([P, 512], bf16, tag=f"at{t}")
                    nc.scalar.activation(
                        a, ps, AF.Exp, bias=mb[:, t:t + 1], scale=SCALE
                    )
                    at.append(a)
                at_all.append(at)

            for ch in range(2):
                at = at_all[ch]

                # softmax row-sums: accumulate the at tiles on the (idle) DVE
                # and partition-reduce with a single f32 ones-matmul instead
                # of 16 PE matmuls.
                acc = accp.tile([P, 512], f32, tag="acc")
                nc.vector.tensor_tensor(acc, at[0], at[1], op=ALU.add)
                for t in range(2, L // P):
                    nc.vector.tensor_tensor(acc, acc, at[t], op=ALU.add)
                sp = pssum.tile([1, 512], f32, tag="sum")
                nc.tensor.matmul(sp, ones_c, acc, start=True, stop=True)
                sums = smp.tile([1, 512], f32, tag="sums")
                nc.vector.tensor_copy(sums, sp)
                rc = []
                for s in range(4):
                    tp = pstp.tile([P, 1], f32, tag="tp")
                    nc.tensor.transpose(tp, sums[0:1, s * P:(s + 1) * P], ident1)
                    r = smp.tile([P, 1], f32, tag=f"rc{s}", name=f"rc{s}")
                    nc.vector.reciprocal(r, tp)
                    rc.append(r)

                # Z^T = sum_key v[key, dblk] * attn^T[key, q]
                z_sb = []
                for dblk in range(V_F // P):
                    pz = psz.tile([P, 512], f32, tag="z")
                    for t in range(L // P):
                        nc.tensor.matmul(
                            pz, v_sb[t][:, dblk * P:(dblk + 1) * P], at[t],
                            start=(t == 0), stop=(t == L // P - 1),
                        )
                    z = zsp.tile([P, 512], bf16, tag=f"z{dblk}", name=f"z{dblk}")
                    nc.vector.tensor_copy(z, pz)
                    z_sb.append(z)

                # output projection + deferred normalization + bias.
                # nch inner next to the shared z lhsT so consecutive matmuls
                # reuse the stationary operand (half the LDWEIGHTS).
                for s in range(4):
                    r0 = ch * 512 + s * P
                    pys = [psy.tile([P, 512], f32, tag=f"y{j}", name=f"py{j}")
                           for j in range(2)]
                    for dblk in range(V_F // P):
                        for nch in range(2):
                            nc.tensor.matmul(
                                pys[nch],
                                z_sb[dblk][:, s * P:(s + 1) * P],
                                wo_sb[dblk][:, nch * 512:(nch + 1) * 512],
                                start=(dblk == 0), stop=(dblk == V_F // P - 1),
                            )
                    for nch in range(2):
                        ysb = ysp.tile([P, 512], f32, tag="ysb")
                        nc.vector.scalar_tensor_tensor(
                            ysb, pys[nch], rc[s], bob[:, nch * 512:(nch + 1) * 512],
                            op0=ALU.mult, op1=ALU.add,
                        )
                        nc.sync.dma_start(
                            y_d[r0:r0 + P, nch * 512:(nch + 1) * 512], ysb
                        )

    with tile.TileContext(nc) as tc:
        for r in range(reps):
            if r:
                tc.strict_bb_all_engine_barrier()
            with ExitStack() as ctx:
                _emit_body(tc, ctx)

    nc.compile()
    return nc


def get_nc(reps=1):
    key = f"nc{reps}"
    if key not in _NC_CACHE:
        _NC_CACHE[key] = _build_program(reps)
    return _NC_CACHE[key]


def prepare_in_maps(inputs):
    bf = ml_dtypes.bfloat16
    f32 = np.float32

    q_real = np.asarray(inputs["q_real"], f32)
    q_imag = np.asarray(inputs["q_imag"], f32)
    k_real = np.asarray(inputs["k_real"], f32)
    k_imag = np.asarray(inputs["k_imag"], f32)
    v_real = np.asarray(inputs["v_real"], f32)
    v_imag = np.asarray(inputs["v_imag"], f32)
    pad_mask = np.asarray(inputs["pad_mask"]).astype(bool)
    W_qkv = np.asarray(inputs["W_qkv"], f32)
    b_qkv = np.asarray(inputs["b_qkv"], f32)
    W_out = np.asarray(inputs["W_out"], f32)
    b_out = np.asarray(inputs["b_out"], f32)

    sel_qk = np.r_[0:D, 2 * D:3 * D]          # q_real + k_real output blocks
    wqkT = np.ascontiguousarray(W_qkv[sel_qk, :].T.astype(bf))
    wvT = np.ascontiguousarray(W_qkv[4 * D:6 * D, :].T.astype(bf))
    woT = np.ascontiguousarray(W_out.T.astype(bf))
    bqk = np.ascontiguousarray(b_qkv[sel_qk].reshape(QK_F // P, P).T.astype(f32))
    bvb = np.ascontiguousarray(np.broadcast_to(b_qkv[4 * D:6 * D], (P, V_F)).astype(f32))
    bob = np.ascontiguousarray(np.broadcast_to(b_out, (P, V_F)).astype(f32))

    x = np.concatenate([q_real, q_imag, k_real, k_imag, v_real, v_imag], axis=-1)

    in_maps = []
    for c in range(NCORES):
        b, h = divmod(c, 2)
        xT = np.ascontiguousarray(x[b][h * HALF:(h + 1) * HALF].T.astype(bf))
        # -ln(4): scales the unnormalized exp weights into comfortable fp8e4
        # range; cancels exactly in the softmax normalization.
        mbias = np.where(pad_mask[b], f32(NEG), f32(-1.3862944))
        mbt = np.ascontiguousarray(mbias.reshape(L // P, P).T.astype(f32))
        in_maps.append({
            "xT": xT, "wqkT": wqkT, "wvT": wvT, "woT": woT,
            "maskb": mbt, "bqk": bqk, "bvb": bvb, "bob": bob,
        })
    return in_maps


def assemble_outputs(results):
    out_real = np.empty((B, L, D), np.float32)
    out_imag = np.empty((B, L, D), np.float32)
    for c in range(NCORES):
        y = np.asarray(results[c]["y"], np.float32)
        b, h = divmod(c, 2)
        out_real[b, h * HALF:(h + 1) * HALF] = y[:, :D]
        out_imag[b, h * HALF:(h + 1) * HALF] = y[:, D:]
    return out_real, out_imag


def _make_executor(reps=1):
    """One jitted SPMD callable per process (mirrors bass2jax.run_bass_via_pjrt
    but is built once and reused, so repeated runs don't recompile)."""
    import jax
    from concourse import bass2jax, mybir

    try:
        jax.config.update("jax_compilation_cache_dir", "/tmp/jax_neff_cache")
        jax.config.update("jax_persistent_cache_min_compile_time_secs", 5.0)
    except Exception:
        pass

    nc = get_nc(reps)
    bass2jax.install_neuronx_cc_hook()
    partition_name = nc.partition_id_tensor.name if nc.partition_id_tensor else None

    in_names, out_names, out_avals, zero_outs = [], [], [], []
    for alloc in nc.m.functions[0].allocations:
        if not isinstance(alloc, mybir.MemoryLocationSet):
            continue
        name = alloc.memorylocations[0].name
        if alloc.kind == "ExternalInput":
            if name != partition_name:
                in_names.append(name)
        elif alloc.kind == "ExternalOutput":
            out_names.append(name)
            shape = tuple(alloc.tensor_shape)
            dtype = mybir.dt.np(alloc.dtype)
            out_avals.append(jax.core.ShapedArray(shape, dtype))
            zero_outs.append((shape, dtype))
    n_params = len(in_names)
    n_outs = len(out_avals)
    all_in_names = list(in_names) + list(out_names)
    if partition_name is not None:
        all_in_names.append(partition_name)

    def _body(*args):
        operands = list(args)
        if partition_name is not None:
            operands.append(bass2jax.partition_id_tensor())
        outs = bass2jax._bass_exec_p.bind(
            *operands,
            out_avals=tuple(out_avals),
            in_names=tuple(all_in_names),
            out_names=tuple(out_names),
            lowering_input_output_aliases=(),
            sim_require_finite=True,
            sim_require_nnan=True,
            nc=nc,
        )
        return tuple(outs)

    devices = jax.devices()[:NCORES]
    assert len(devices) == NCORES
    mesh = bass2jax.Mesh(np.asarray(devices), ("core",))
    in_specs = (bass2jax.PartitionSpec("core"),) * (n_params + n_outs)
    out_specs = (bass2jax.PartitionSpec("core"),) * n_outs
    donate = tuple(range(n_params, n_params + n_outs))
    sharded = jax.jit(
        bass2jax.shard_map(
            _body, mesh=mesh, in_specs=in_specs,
            out_specs=out_specs, check_rep=False,
        ),
        donate_argnums=donate,
        keep_unused=True,
    )
    return {
        "sharded": sharded,
        "mesh": mesh,
        "in_names": in_names,
        "out_names": out_names,
        "out_avals": out_avals,
        "zero_outs": zero_outs,
    }


def get_executor(reps=1):
    key = f"exec{reps}"
    if key not in _NC_CACHE:
        _NC_CACHE[key] = _make_executor(reps)
    return _NC_CACHE[key]


def concat_inputs(in_maps, ex):
    return [
        np.concatenate([np.asarray(in_maps[c][n]) for c in range(NCORES)], axis=0)
        for n in ex["in_names"]
    ]


def make_zero_outs(ex):
    return [
        np.zeros((NCORES * s[0], *s[1:]), d) for (s, d) in ex["zero_outs"]
    ]


def execute(concat_in, ex):
    out_arrs = ex["sharded"](*concat_in, *make_zero_outs(ex))
    results = [
        {
            name: np.asarray(out_arrs[i]).reshape(
                NCORES, *ex["out_avals"][i].shape
            )[c]
            for i, name in enumerate(ex["out_names"])
        }
        for c in range(NCORES)
    ]
    return results


def run(inputs, trace=False):
    from concourse.bass_utils import run_bass_kernel_spmd

    nc = get_nc()
    in_maps = prepare_in_maps(inputs)
    return run_bass_kernel_spmd(
        nc, in_maps, core_ids=list(range(NCORES)), trace=trace
    )


def kernel(**inputs):
    ex = get_executor()
    in_maps = prepare_in_maps(inputs)
    results = execute(concat_inputs(in_maps, ex), ex)
    return assemble_outputs(results)

